# revision 1
# baseline (speedup 1.0000x reference)
"""MultiHeadAttention Trainium2 kernel (8 NeuronCores, Bass/Tile).

Problem: B=2, S=2048, D=1024, H=16, DK=64 fp32 MHA (torch-Linear style
projections, softmax attention, output projection).

Sharding: core c = (batch b = c//4, head-group g = c%4); each core handles
4 heads of one batch, entirely in a transposed layout (features on
partitions, sequence on the free axis):
  qhT/khT  = (W_g x^T + b)       [2 pairs x 128, 2048]
  vh       = x_v Wv_g^T          [2048, 4x65] (ones col -> row sums)
  scoresT  = khT^T qhT           per (pair, ktile, qtile) -> PSUM
  expT     = exp(scoresT/8 - 2)  ACT (bias -2 for fp16 headroom)
  rawT     = vh_aug^T expT       PV matmul; row 64 = softmax denominator
  outT     = rawT[0:64] * (1/rawT[64])
  partialT = woT^T outT          [1024, 2048] fp32 -> DRAM
Host: out[b] = sum_g partialT(b,g)^T + (Wo bv + bo).

PE is exact on fp16/bf16 operands (fp32 accumulate); per-stage operand
dtypes are configurable below. Softmax denominators come free via the
ones column (attention rows sum to 1, which also lets Wo@bv fold into a
host-side constant). No collectives; host sums 4 partials per batch.
"""

import numpy as np

B, S, D, H = 2, 2048, 1024, 16
DK = D // H          # 64
N_CORES = 8
HG = H // 4          # 4 head-groups
HL = 4               # heads per core
FEAT = HL * DK       # 256 per-core features
NQT = S // 512       # 4 query tiles
NKT = S // 128       # 16 key tiles
NDT = D // 128       # 8 contraction tiles (d-model)

# per-stage matmul operand dtypes ("fp16" | "bf16")
DT_QK = "fp16"   # x_q/x_k, Wq/Wk, qhT/khT (score operands)
DT_V = "fp16"    # x_v, Wv
DT_PV = "bf16"   # vh_aug, expT
DT_O = "fp16"    # Wo, outT (feeds final output directly)

_cache = {}


def _np_dt(name):
    if name == "fp16":
        return np.float16
    import ml_dtypes
    return ml_dtypes.bfloat16


def _build():
    import concourse.mybir as mybir
    import concourse.tile as tile
    from concourse import bacc

    fp32 = mybir.dt.float32
    dt_qk = getattr(mybir.dt, "float16" if DT_QK == "fp16" else "bfloat16")
    dt_v = getattr(mybir.dt, "float16" if DT_V == "fp16" else "bfloat16")
    dt_pv = getattr(mybir.dt, "float16" if DT_PV == "fp16" else "bfloat16")
    dt_o = getattr(mybir.dt, "float16" if DT_O == "fp16" else "bfloat16")

    nc = bacc.Bacc("TRN2", target_bir_lowering=False, debug=False,
                   num_devices=N_CORES)

    xqT = nc.dram_tensor("xqT", [D, S], dt_qk, kind="ExternalInput").ap()
    xkT = nc.dram_tensor("xkT", [D, S], dt_qk, kind="ExternalInput").ap()
    xvT = nc.dram_tensor("xvT", [D, S], dt_v, kind="ExternalInput").ap()
    wqT = nc.dram_tensor("wqT", [D, FEAT], dt_qk, kind="ExternalInput").ap()
    wkT = nc.dram_tensor("wkT", [D, FEAT], dt_qk, kind="ExternalInput").ap()
    wvT = nc.dram_tensor("wvT", [D, FEAT], dt_v, kind="ExternalInput").ap()
    woT = nc.dram_tensor("woT", [FEAT, D], dt_o, kind="ExternalInput").ap()
    bq2 = nc.dram_tensor("bq2", [FEAT, 1], fp32, kind="ExternalInput").ap()
    bk2 = nc.dram_tensor("bk2", [FEAT, 1], fp32, kind="ExternalInput").ap()
    out_d = nc.dram_tensor("partialT", [D, S], fp32, kind="ExternalOutput").ap()

    xq_r = xqT.rearrange("(t p) s -> p t s", p=128)
    xk_r = xkT.rearrange("(t p) s -> p t s", p=128)
    xv_r = xvT.rearrange("(t p) s -> p t s", p=128)

    with tile.TileContext(nc) as tc:
        with (
            tc.tile_pool(name="xin", bufs=1) as xin,
            tc.tile_pool(name="win", bufs=1) as win,
            tc.tile_pool(name="proj", bufs=1) as proj,
            tc.tile_pool(name="pexp", bufs=17) as pexp,
            tc.tile_pool(name="pout", bufs=4) as pout,
            tc.tile_pool(name="pnrm", bufs=2) as pnrm,
            tc.tile_pool(name="pp", bufs=2, space="PSUM") as pp,
            tc.tile_pool(name="ps2", bufs=2, space="PSUM") as ps2,
            tc.tile_pool(name="pspv", bufs=2, space="PSUM") as pspv,
        ):
            # ---- load inputs: weights first (small, unblock compute),
            # then x per d-tile in consumption order (v, then q/k) ----
            wq3 = win.tile([128, NDT, FEAT], dt_qk, tag="wq")
            wk3 = win.tile([128, NDT, FEAT], dt_qk, tag="wk")
            wv3 = win.tile([128, NDT, FEAT], dt_v, tag="wv")
            wo3 = win.tile([128, 2, D], dt_o, tag="wo")
            bq3 = win.tile([128, 2, 1], fp32, tag="bq")
            bk3 = win.tile([128, 2, 1], fp32, tag="bk")
            nc.sync.dma_start(wk3[:], wkT.rearrange("(t p) f -> p t f", p=128))
            nc.sync.dma_start(wq3[:], wqT.rearrange("(t p) f -> p t f", p=128))
            nc.sync.dma_start(wv3[:], wvT.rearrange("(t p) f -> p t f", p=128))
            nc.sync.dma_start(wo3[:], woT.rearrange("(t p) j -> p t j", p=128))
            nc.sync.dma_start(bq3[:], bq2.rearrange("(t p) o -> p t o", p=128))
            nc.sync.dma_start(bk3[:], bk2.rearrange("(t p) o -> p t o", p=128))
            xq3 = xin.tile([128, NDT, S], dt_qk, tag="xq")
            xk3 = xin.tile([128, NDT, S], dt_qk, tag="xk")
            xv3 = xin.tile([128, NDT, S], dt_v, tag="xv")
            for t in range(NDT):
                nc.sync.dma_start(xk3[:, t, :], xk_r[:, t, :])
                nc.sync.dma_start(xq3[:, t, :], xq_r[:, t, :])
            for t in range(NDT):
                nc.sync.dma_start(xv3[:, t, :], xv_r[:, t, :])

            # ---- persistent intermediates ----
            qh3 = proj.tile([128, 2, S], dt_qk, tag="qh")   # pair-packed
            kh3 = proj.tile([128, 2, S], dt_qk, tag="kh")
            vha = proj.tile([128, NKT, HL, DK + 1], dt_pv, tag="vha")
            ot3 = proj.tile([128, 2, S], dt_o, tag="outT")

            nc.gpsimd.memset(vha[:, :, :, DK], 1.0)  # ones column
            # exp bias -2: headroom under fp16 max (cancels in division)
            ebias = win.tile([128, 1], fp32, tag="ebias")
            nc.gpsimd.memset(ebias[:], -2.0)

            # ---- projections ----
            # emission order: q/k for pair 0 first, then v, then q/k pair 1 —
            # pair-0 scores/exp become schedulable early, keeping ACT busy
            # while the remaining projections still occupy the PE.
            def qk_proj(m):
                for x3, w3, b3, dst in ((xk3, wk3, bk3, kh3),
                                        (xq3, wq3, bq3, qh3)):
                    for n in range(NQT):
                        ps = pp.tile([128, 512], fp32, tag="acc")
                        for kt in range(NDT):
                            nc.tensor.matmul(
                                ps[:],
                                w3[:, kt, m * 128:(m + 1) * 128],
                                x3[:, kt, n * 512:(n + 1) * 512],
                                start=(kt == 0), stop=(kt == NDT - 1))
                        nc.vector.tensor_scalar_add(
                            dst[:, m, n * 512:(n + 1) * 512], ps[:], b3[:, m, :])

            def v_proj():
                for st in range(NKT):
                    ps = pp.tile([128, 256], fp32, tag="acc")
                    for kt in range(NDT):
                        nc.tensor.matmul(
                            ps[:], xv3[:, kt, st * 128:(st + 1) * 128],
                            wv3[:, kt, :],
                            start=(kt == 0), stop=(kt == NDT - 1))
                    nc.vector.tensor_copy(vha[:, st, :, 0:DK], ps[:])

            # ---- attention (split so scores/exp of (0,0) can be
            # emitted before v-proj and qk_proj(1), starting ACT ~35us
            # earlier; PV readers are emitted only after v-proj writes) ----
            def attn_scores(qt, hp):
                e2s = []
                for kt in range(NKT):
                    s2 = ps2.tile([128, 1024], fp32, tag="s2")
                    nc.tensor.matmul(
                        s2[:, 0:512],
                        kh3[0:64, hp, kt * 128:(kt + 1) * 128],
                        qh3[0:64, hp, qt * 512:(qt + 1) * 512],
                        start=True, stop=True)
                    nc.tensor.matmul(
                        s2[:, 512:1024],
                        kh3[64:128, hp, kt * 128:(kt + 1) * 128],
                        qh3[64:128, hp, qt * 512:(qt + 1) * 512],
                        start=True, stop=True)
                    e2 = pexp.tile([128, 1024], dt_pv, tag="e2")
                    if DT_PV == "bf16":   # bf16 range: no overflow risk
                        nc.scalar.activation(
                            e2[:], s2[:],
                            mybir.ActivationFunctionType.Exp, scale=0.125)
                    else:
                        nc.scalar.activation(
                            e2[:], s2[:],
                            mybir.ActivationFunctionType.Exp,
                            scale=0.125, bias=ebias[:])
                    e2s.append(e2)
                return e2s

            def attn_pv(qt, hp, e2s):
                pva = pspv.tile([DK + 1, 512], fp32, tag="pv")
                pvb = pspv.tile([DK + 1, 512], fp32, tag="pv")
                for kt in range(NKT):
                    nc.tensor.matmul(
                        pva[:], vha[:, kt, 2 * hp, :], e2s[kt][:, 0:512],
                        start=(kt == 0), stop=(kt == NKT - 1))
                    nc.tensor.matmul(
                        pvb[:], vha[:, kt, 2 * hp + 1, :],
                        e2s[kt][:, 512:1024],
                        start=(kt == 0), stop=(kt == NKT - 1))
                for pv, half in ((pva, 0), (pvb, 1)):
                    # custom DVE ops must read SBUF, not PSUM
                    srow = pnrm.tile([1, 512], fp32, tag="srow")
                    nc.vector.tensor_copy(srow[:], pv[DK:DK + 1, :])
                    inv = pnrm.tile([1, 512], fp32, tag="inv")
                    nc.vector.reciprocal_approx_fast(inv[:], srow[:])
                    invb = pnrm.tile([64, 512], fp32, tag="invb")
                    nc.gpsimd.partition_broadcast(invb[:], inv[:])
                    nc.vector.tensor_tensor(
                        ot3[half * 64:(half + 1) * 64, hp,
                            qt * 512:(qt + 1) * 512],
                        pv[0:DK, :], invb[:], mybir.AluOpType.mult)

            def oproj(qt):
                for jt in range(NDT):
                    ps = pp.tile([128, 512], fp32, tag="acc")
                    for m in range(2):
                        nc.tensor.matmul(
                            ps[:], wo3[:, m, jt * 128:(jt + 1) * 128],
                            ot3[:, m, qt * 512:(qt + 1) * 512],
                            start=(m == 0), stop=(m == 1))
                    po = pout.tile([128, 512], fp32, tag="po")
                    nc.vector.tensor_copy(po[:], ps[:])
                    nc.sync.dma_start(
                        out_d[jt * 128:(jt + 1) * 128,
                              qt * 512:(qt + 1) * 512], po[:])

            qk_proj(0)
            e00 = attn_scores(0, 0)   # ACT starts here, during qk1/v-proj
            qk_proj(1)
            v_proj()
            attn_pv(0, 0, e00)
            e01 = attn_scores(0, 1)
            attn_pv(0, 1, e01)
            oproj(0)
            for qt in range(1, NQT):
                for hp in range(2):
                    e = attn_scores(qt, hp)
                    attn_pv(qt, hp, e)
                oproj(qt)

    nc.compile()
    return nc


def kernel(q, k, v, Wq, bq, Wk, bk, Wv, bv, Wo, bo, _trace=False):
    from concourse import bass_utils

    if "nc" not in _cache:
        _cache["nc"] = _build()
    nc = _cache["nc"]

    q = np.asarray(q, np.float32)
    k = np.asarray(k, np.float32)
    v = np.asarray(v, np.float32)
    Wq = np.asarray(Wq, np.float32)
    Wk = np.asarray(Wk, np.float32)
    Wv = np.asarray(Wv, np.float32)
    Wo = np.asarray(Wo, np.float32)
    bq = np.asarray(bq, np.float32)
    bk = np.asarray(bk, np.float32)
    bv = np.asarray(bv, np.float32)
    bo = np.asarray(bo, np.float32)

    d_qk, d_v, d_o = _np_dt(DT_QK), _np_dt(DT_V), _np_dt(DT_O)
    xT = {}
    for b in range(B):
        xT[("q", b)] = np.ascontiguousarray(q[b].T).astype(d_qk)
        xT[("k", b)] = np.ascontiguousarray(k[b].T).astype(d_qk)
        xT[("v", b)] = np.ascontiguousarray(v[b].T).astype(d_v)
    wT = {}
    for g in range(HG):
        sl = slice(g * FEAT, (g + 1) * FEAT)
        wT[("q", g)] = np.ascontiguousarray(Wq[sl, :].T).astype(d_qk)
        wT[("k", g)] = np.ascontiguousarray(Wk[sl, :].T).astype(d_qk)
        wT[("v", g)] = np.ascontiguousarray(Wv[sl, :].T).astype(d_v)
        wT[("o", g)] = np.ascontiguousarray(Wo[:, sl].T).astype(d_o)

    in_maps = []
    for c in range(N_CORES):
        b, g = divmod(c, HG)
        sl = slice(g * FEAT, (g + 1) * FEAT)
        in_maps.append({
            "xqT": xT[("q", b)], "xkT": xT[("k", b)], "xvT": xT[("v", b)],
            "wqT": wT[("q", g)], "wkT": wT[("k", g)], "wvT": wT[("v", g)],
            "woT": wT[("o", g)],
            "bq2": np.ascontiguousarray(bq[sl]).reshape(FEAT, 1),
            "bk2": np.ascontiguousarray(bk[sl]).reshape(FEAT, 1),
        })

    kwargs = {}
    if _trace:
        _install_profile_shim()
        kwargs = dict(trace=True, trace_cores=list(range(N_CORES)))
    res = bass_utils.run_bass_kernel_spmd(
        nc, in_maps, core_ids=list(range(N_CORES)), **kwargs)
    _cache["last_results"] = res

    final_bias = (Wo @ bv + bo).astype(np.float32)  # attn rows sum to 1
    out = np.empty((B, S, D), np.float32)
    for b in range(B):
        acc = res.results[b * HG]["partialT"].copy()
        for g in range(1, HG):
            acc += res.results[b * HG + g]["partialT"]
        out[b] = acc.T + final_bias
    return out


def _install_profile_shim():
    """Provide antenv.axon_hooks so trace=True works under axon."""
    import sys
    import types

    import antenv

    if "antenv.axon_hooks" in sys.modules:
        return
    mod = types.ModuleType("antenv.axon_hooks")
    mod._hook = None
    mod.set_axon_ntff_profile_hook = lambda h: setattr(mod, "_hook", h)
    mod.get_axon_ntff_profile_hook = lambda: mod._hook
    sys.modules["antenv.axon_hooks"] = mod
    antenv.axon_hooks = mod
    try:
        from trn_agent_boot.trn_boot import _ntff_profile_via_ctypes
        mod.set_axon_ntff_profile_hook(
            _ntff_profile_via_ctypes("/opt/axon/libaxon_pjrt.so"))
    except Exception:
        pass



# revision 3
# speedup vs baseline: 1.0660x; 1.0660x over previous
"""MultiHeadAttention Trainium2 kernel (8 NeuronCores, Bass/Tile).

Problem: B=2, S=2048, D=1024, H=16, DK=64 fp32 MHA (torch-Linear style
projections, softmax attention, output projection).

Sharding: core c = (batch b = c//4, head-group g = c%4); each core handles
4 heads of one batch, entirely in a transposed layout (features on
partitions, sequence on the free axis):
  qhT/khT  = (W_g x^T + b)       [2 pairs x 128, 2048]
  vh       = x_v Wv_g^T          [2048, 4x65] (ones col -> row sums)
  scoresT  = khT^T qhT           per (pair, ktile, qtile) -> PSUM
  expT     = exp(scoresT/8)      ACT -> bf16
  rawT     = vh_aug^T expT       PV matmul; row 64 = softmax denominator
  outT     = rawT[0:64] * (1/rawT[64])
  partialT = woT^T outT          [1024, 2048] fp16 -> DRAM
Host: out[b] = sum_g partialT(b,g)^T + (Wo bv + bo).

v2 pipeline notes (v1 measured 257us, PE 75% busy):
- input DMA rings are serviced round-robin, so unordered loads all land
  at ~21us; ring chaining (chain_iter_dep) staggers wk->wq->xk->xq->xv
  so k-proj starts at ~7us.
- warmup matmuls ramp the PE out of its low p-state during the DMA wait.
- e2 exp tiles cycle through the same 32KB pool slots as the (dead by
  then) xk/xq/xv input tiles, giving 2 units of exp/PV pipelining
  without exceeding SBUF.
- partial output written fp16 (halves tail DMA); host sums in fp32.
"""

import numpy as np

B, S, D, H = 2, 2048, 1024, 16
DK = D // H          # 64
N_CORES = 8
HG = H // 4          # 4 head-groups
HL = 4               # heads per core
FEAT = HL * DK       # 256 per-core features
NQT = S // 512       # 4 query tiles
NKT = S // 128       # 16 key tiles
NDT = D // 128       # 8 contraction tiles (d-model)

DT_QK = "fp16"   # x_q/x_k, Wq/Wk, qhT/khT (score operands)
DT_V = "fp16"    # x_v, Wv
DT_PV = "bf16"   # vh_aug, expT
DT_O = "fp16"    # Wo, outT
N_WARMUP = 28    # PE p-state warmup matmuls during initial DMA wait

_cache = {}


def _np_dt(name):
    if name == "fp16":
        return np.float16
    import ml_dtypes
    return ml_dtypes.bfloat16


def _build():
    import concourse.mybir as mybir
    import concourse.tile as tile
    from concourse import bacc

    fp32 = mybir.dt.float32
    dt_qk = getattr(mybir.dt, "float16" if DT_QK == "fp16" else "bfloat16")
    dt_v = getattr(mybir.dt, "float16" if DT_V == "fp16" else "bfloat16")
    dt_pv = getattr(mybir.dt, "float16" if DT_PV == "fp16" else "bfloat16")
    dt_o = getattr(mybir.dt, "float16" if DT_O == "fp16" else "bfloat16")
    dt_out = mybir.dt.float16

    nc = bacc.Bacc("TRN2", target_bir_lowering=False, debug=False,
                   num_devices=N_CORES)

    xqT = nc.dram_tensor("xqT", [D, S], dt_qk, kind="ExternalInput").ap()
    xkT = nc.dram_tensor("xkT", [D, S], dt_qk, kind="ExternalInput").ap()
    xvT = nc.dram_tensor("xvT", [D, S], dt_v, kind="ExternalInput").ap()
    wqT = nc.dram_tensor("wqT", [D, FEAT], dt_qk, kind="ExternalInput").ap()
    wkT = nc.dram_tensor("wkT", [D, FEAT], dt_qk, kind="ExternalInput").ap()
    wvT = nc.dram_tensor("wvT", [D, FEAT], dt_v, kind="ExternalInput").ap()
    woT = nc.dram_tensor("woT", [FEAT, D], dt_o, kind="ExternalInput").ap()
    bq2 = nc.dram_tensor("bq2", [FEAT, 1], fp32, kind="ExternalInput").ap()
    bk2 = nc.dram_tensor("bk2", [FEAT, 1], fp32, kind="ExternalInput").ap()
    out_d = nc.dram_tensor("partialT", [D, S], dt_out,
                           kind="ExternalOutput").ap()

    xq_r = xqT.rearrange("(t p) s -> p t s", p=128)
    xk_r = xkT.rearrange("(t p) s -> p t s", p=128)
    xv_r = xvT.rearrange("(t p) s -> p t s", p=128)

    with tile.TileContext(nc) as tc:
        def chain(inst):
            # serialize DMA rings so early tensors get full HBM bandwidth
            try:
                tc.chain_iter_dep("dmachain", inst)
            except Exception:
                pass

        with (
            tc.tile_pool(name="win", bufs=1) as win,
            tc.tile_pool(name="big", bufs=4) as big,
            tc.tile_pool(name="proj", bufs=1) as proj,
            tc.tile_pool(name="pout", bufs=4) as pout,
            tc.tile_pool(name="pnrm", bufs=2) as pnrm,
            tc.tile_pool(name="pp", bufs=2, space="PSUM") as pp,
            tc.tile_pool(name="ps2", bufs=2, space="PSUM") as ps2,
            tc.tile_pool(name="pspv", bufs=2, space="PSUM") as pspv,
        ):
            # ---- DMA: biases unchained (tiny), then one chained stream in
            # consumption order; each ring gets full bandwidth in turn ----
            wq3 = win.tile([128, NDT, FEAT], dt_qk, tag="wq")
            wk3 = win.tile([128, NDT, FEAT], dt_qk, tag="wk")
            wv3 = win.tile([128, NDT, FEAT], dt_v, tag="wv")
            wo3 = win.tile([128, 2, D], dt_o, tag="wo")
            bq3 = win.tile([128, 2, 1], fp32, tag="bq")
            bk3 = win.tile([128, 2, 1], fp32, tag="bk")
            nc.sync.dma_start(bk3[:], bk2.rearrange("(t p) o -> p t o", p=128))
            nc.sync.dma_start(bq3[:], bq2.rearrange("(t p) o -> p t o", p=128))

            xk3 = big.tile([128, NDT, S], dt_qk, tag="big")
            xq3 = big.tile([128, NDT, S], dt_qk, tag="big")
            xv3 = big.tile([128, NDT, S], dt_v, tag="big")

            chain(nc.sync.dma_start(
                wk3[:], wkT.rearrange("(t p) f -> p t f", p=128)))
            chain(nc.sync.dma_start(
                wq3[:], wqT.rearrange("(t p) f -> p t f", p=128)))
            for x3, xr in ((xk3, xk_r), (xq3, xq_r), (xv3, xv_r)):
                for t0 in (0, 4):
                    chain(nc.sync.dma_start(
                        x3[:, t0:t0 + 4, :], xr[:, t0:t0 + 4, :]))
            chain(nc.sync.dma_start(
                wv3[:], wvT.rearrange("(t p) f -> p t f", p=128)))
            chain(nc.sync.dma_start(
                wo3[:], woT.rearrange("(t p) j -> p t j", p=128)))

            # ---- persistent intermediates ----
            qh3 = proj.tile([128, 2, S], dt_qk, tag="qh")   # pair-packed
            kh3 = proj.tile([128, 2, S], dt_qk, tag="kh")
            vha = proj.tile([128, NKT, HL, DK + 1], dt_pv, tag="vha")
            ot3 = proj.tile([128, 2, S], dt_o, tag="outT")
            nc.gpsimd.memset(vha[:, :, :, DK], 1.0)  # ones col -> denominators

            # ---- PE p-state warmup + ACT exp-table preload while the first
            # DMAs land: matmuls on a zeroed tile into one psum scratch ----
            wdum = win.tile([128, 512], dt_qk, tag="wdum")
            junk = win.tile([128, 512], fp32, tag="junk")
            nc.gpsimd.memset(wdum[:], 0.0)
            nc.scalar.activation(junk[0:1, 0:1], wdum[0:1, 0:1],
                                 mybir.ActivationFunctionType.Exp, scale=1.0)
            wu = pp.tile([128, 512], fp32, tag="acc")
            for i in range(N_WARMUP):
                nc.tensor.matmul(wu[:], wdum[:, 0:128], wdum[:],
                                 start=(i == 0), stop=(i == N_WARMUP - 1))
            nc.vector.tensor_copy(junk[:], wu[:])

            # ---- projections: 2 psum accumulators per pass, kt-interleaved
            # so matmuls chase the chunked x DMAs ----
            def qk_pass(x3, w3, b3, dst, m, nn):
                accs = [pp.tile([128, 512], fp32, tag="acc", name=f"acc{n}")
                        for n in nn]
                for kt in range(NDT):
                    for a, n in zip(accs, nn):
                        nc.tensor.matmul(
                            a[:], w3[:, kt, m * 128:(m + 1) * 128],
                            x3[:, kt, n * 512:(n + 1) * 512],
                            start=(kt == 0), stop=(kt == NDT - 1))
                for a, n in zip(accs, nn):
                    nc.vector.tensor_scalar_add(
                        dst[:, m, n * 512:(n + 1) * 512], a[:], b3[:, m, :])

            def v_proj():
                for st in range(NKT):
                    ps = pp.tile([128, 512], fp32, tag="acc")
                    for kt in range(NDT):
                        nc.tensor.matmul(
                            ps[:, 0:256], xv3[:, kt, st * 128:(st + 1) * 128],
                            wv3[:, kt, :],
                            start=(kt == 0), stop=(kt == NDT - 1))
                    nc.vector.tensor_copy(vha[:, st, :, 0:DK], ps[:, 0:256])

            def attn_scores(qt, hp, e2u):
                for kt in range(NKT):
                    s2 = ps2.tile([128, 1024], fp32, tag="s2")
                    nc.tensor.matmul(
                        s2[:, 0:512],
                        kh3[0:64, hp, kt * 128:(kt + 1) * 128],
                        qh3[0:64, hp, qt * 512:(qt + 1) * 512],
                        start=True, stop=True)
                    nc.tensor.matmul(
                        s2[:, 512:1024],
                        kh3[64:128, hp, kt * 128:(kt + 1) * 128],
                        qh3[64:128, hp, qt * 512:(qt + 1) * 512],
                        start=True, stop=True)
                    nc.scalar.activation(
                        e2u[:, kt, :], s2[:],
                        mybir.ActivationFunctionType.Exp, scale=0.125)

            def attn_pv(qt, hp, e2u):
                pva = pspv.tile([DK + 1, 512], fp32, tag="pv")
                pvb = pspv.tile([DK + 1, 512], fp32, tag="pv")
                for kt in range(NKT):
                    nc.tensor.matmul(
                        pva[:], vha[:, kt, 2 * hp, :], e2u[:, kt, 0:512],
                        start=(kt == 0), stop=(kt == NKT - 1))
                    nc.tensor.matmul(
                        pvb[:], vha[:, kt, 2 * hp + 1, :],
                        e2u[:, kt, 512:1024],
                        start=(kt == 0), stop=(kt == NKT - 1))
                for pv, half in ((pva, 0), (pvb, 1)):
                    # custom DVE ops must read SBUF, not PSUM
                    srow = pnrm.tile([1, 512], fp32, tag="srow")
                    nc.vector.tensor_copy(srow[:], pv[DK:DK + 1, :])
                    inv = pnrm.tile([1, 512], fp32, tag="inv")
                    nc.vector.reciprocal_approx_fast(inv[:], srow[:])
                    invb = pnrm.tile([64, 512], fp32, tag="invb")
                    nc.gpsimd.partition_broadcast(invb[:], inv[:])
                    nc.vector.tensor_tensor(
                        ot3[half * 64:(half + 1) * 64, hp,
                            qt * 512:(qt + 1) * 512],
                        pv[0:DK, :], invb[:], mybir.AluOpType.mult)

            def oproj(qt):
                for jt in range(NDT):
                    ps = pp.tile([128, 512], fp32, tag="acc")
                    for m in range(2):
                        nc.tensor.matmul(
                            ps[:], wo3[:, m, jt * 128:(jt + 1) * 128],
                            ot3[:, m, qt * 512:(qt + 1) * 512],
                            start=(m == 0), stop=(m == 1))
                    po = pout.tile([128, 512], dt_out, tag="po")
                    nc.vector.tensor_copy(po[:], ps[:])
                    nc.sync.dma_start(
                        out_d[jt * 128:(jt + 1) * 128,
                              qt * 512:(qt + 1) * 512], po[:])

            def e2tile(name):
                return big.tile([128, NKT, 1024], dt_pv, tag="big", name=name)

            # ---- emission order == per-engine execution order ----
            for m in range(2):                       # k-proj (all pairs)
                for nn in ((0, 1), (2, 3)):
                    qk_pass(xk3, wk3, bk3, kh3, m, nn)
            qk_pass(xq3, wq3, bq3, qh3, 0, (0,))     # q-proj heads only
            qk_pass(xq3, wq3, bq3, qh3, 1, (0,))
            e00 = e2tile("e00")
            attn_scores(0, 0, e00)                   # ACT starts here
            e01 = e2tile("e01")
            attn_scores(0, 1, e01)
            for m in range(2):                       # q-proj remainder
                qk_pass(xq3, wq3, bq3, qh3, m, (1, 2))
                qk_pass(xq3, wq3, bq3, qh3, m, (3,))
            v_proj()
            attn_pv(0, 0, e00)
            e10 = e2tile("e10")
            attn_scores(1, 0, e10)
            attn_pv(0, 1, e01)
            oproj(0)
            prev = {(1, 0): e10}
            for qt in range(1, NQT):
                e_b = e2tile(f"e{qt}1")
                attn_scores(qt, 1, e_b)
                attn_pv(qt, 0, prev[(qt, 0)])
                if qt < NQT - 1:
                    e_a = e2tile(f"e{qt + 1}0")
                    attn_scores(qt + 1, 0, e_a)
                    prev[(qt + 1, 0)] = e_a
                attn_pv(qt, 1, e_b)
                oproj(qt)

    nc.compile()
    return nc


def kernel(q, k, v, Wq, bq, Wk, bk, Wv, bv, Wo, bo, _trace=False):
    from concourse import bass_utils

    if "nc" not in _cache:
        _cache["nc"] = _build()
    nc = _cache["nc"]

    q = np.asarray(q, np.float32)
    k = np.asarray(k, np.float32)
    v = np.asarray(v, np.float32)
    Wq = np.asarray(Wq, np.float32)
    Wk = np.asarray(Wk, np.float32)
    Wv = np.asarray(Wv, np.float32)
    Wo = np.asarray(Wo, np.float32)
    bq = np.asarray(bq, np.float32)
    bk = np.asarray(bk, np.float32)
    bv = np.asarray(bv, np.float32)
    bo = np.asarray(bo, np.float32)

    d_qk, d_v, d_o = _np_dt(DT_QK), _np_dt(DT_V), _np_dt(DT_O)
    xT = {}
    for b in range(B):
        xT[("q", b)] = np.ascontiguousarray(q[b].T).astype(d_qk)
        xT[("k", b)] = np.ascontiguousarray(k[b].T).astype(d_qk)
        xT[("v", b)] = np.ascontiguousarray(v[b].T).astype(d_v)
    wT = {}
    for g in range(HG):
        sl = slice(g * FEAT, (g + 1) * FEAT)
        wT[("q", g)] = np.ascontiguousarray(Wq[sl, :].T).astype(d_qk)
        wT[("k", g)] = np.ascontiguousarray(Wk[sl, :].T).astype(d_qk)
        wT[("v", g)] = np.ascontiguousarray(Wv[sl, :].T).astype(d_v)
        wT[("o", g)] = np.ascontiguousarray(Wo[:, sl].T).astype(d_o)

    in_maps = []
    for c in range(N_CORES):
        b, g = divmod(c, HG)
        sl = slice(g * FEAT, (g + 1) * FEAT)
        in_maps.append({
            "xqT": xT[("q", b)], "xkT": xT[("k", b)], "xvT": xT[("v", b)],
            "wqT": wT[("q", g)], "wkT": wT[("k", g)], "wvT": wT[("v", g)],
            "woT": wT[("o", g)],
            "bq2": np.ascontiguousarray(bq[sl]).reshape(FEAT, 1),
            "bk2": np.ascontiguousarray(bk[sl]).reshape(FEAT, 1),
        })

    kwargs = {}
    if _trace:
        _install_profile_shim()
        kwargs = dict(trace=True, trace_cores=list(range(N_CORES)))
    res = bass_utils.run_bass_kernel_spmd(
        nc, in_maps, core_ids=list(range(N_CORES)), **kwargs)
    _cache["last_results"] = res

    final_bias = (Wo @ bv + bo).astype(np.float32)  # attn rows sum to 1
    out = np.empty((B, S, D), np.float32)
    for b in range(B):
        acc = res.results[b * HG]["partialT"].astype(np.float32)
        for g in range(1, HG):
            acc += res.results[b * HG + g]["partialT"].astype(np.float32)
        out[b] = acc.T + final_bias
    return out


def _install_profile_shim():
    """Provide antenv.axon_hooks so trace=True works under axon."""
    import sys
    import types

    import antenv

    if "antenv.axon_hooks" in sys.modules:
        return
    mod = types.ModuleType("antenv.axon_hooks")
    mod._hook = None
    mod.set_axon_ntff_profile_hook = lambda h: setattr(mod, "_hook", h)
    mod.get_axon_ntff_profile_hook = lambda: mod._hook
    sys.modules["antenv.axon_hooks"] = mod
    antenv.axon_hooks = mod
    try:
        from trn_agent_boot.trn_boot import _ntff_profile_via_ctypes
        mod.set_axon_ntff_profile_hook(
            _ntff_profile_via_ctypes("/opt/axon/libaxon_pjrt.so"))
    except Exception:
        pass


# revision 10
# speedup vs baseline: 1.1541x; 1.0826x over previous
"""MultiHeadAttention Trainium2 kernel (8 NeuronCores, Bass/Tile).

Problem: B=2, S=2048, D=1024, H=16, DK=64 fp32 MHA (torch-Linear style
projections, softmax attention, output projection).

Sharding: core c = (batch b = c//4, head-group g = c%4); each core handles
4 heads of one batch, entirely in a transposed layout (features on
partitions, sequence on the free axis):
  qhT/khT  = (W_g x^T + b)       [2 pairs x 128, 2048]
  vh       = x_v Wv_g^T          [2048, 4x65] (ones col -> row sums)
  scoresT  = khT^T qhT           per (pair, ktile, qtile) -> PSUM
  expT     = exp(scoresT/8)      ACT -> bf16
  rawT     = vh_aug^T expT       PV matmul; row 64 = softmax denominator
  outT     = rawT[0:64] * (1/rawT[64])
  partialT = woT^T outT          [1024, 2048] fp16 -> DRAM
Host: out[b] = sum_g partialT(b,g)^T + (Wo bv + bo).

v2 pipeline notes (v1 measured 257us, PE 75% busy):
- input DMA rings are serviced round-robin, so unordered loads all land
  at ~21us; ring chaining (chain_iter_dep) staggers wk->wq->xk->xq->xv
  so k-proj starts at ~7us.
- warmup matmuls ramp the PE out of its low p-state during the DMA wait.
- e2 exp tiles cycle through the same 32KB pool slots as the (dead by
  then) xk/xq/xv input tiles, giving 2 units of exp/PV pipelining
  without exceeding SBUF.
- partial output written fp16 (halves tail DMA); host sums in fp32.
"""

import numpy as np

B, S, D, H = 2, 2048, 1024, 16
DK = D // H          # 64
N_CORES = 8
HG = H // 4          # 4 head-groups
HL = 4               # heads per core
FEAT = HL * DK       # 256 per-core features
NQT = S // 512       # 4 query tiles
NKT = S // 128       # 16 key tiles
NDT = D // 128       # 8 contraction tiles (d-model)

DT_QK = "fp16"   # x_q/x_k, Wq/Wk, qhT/khT (score operands)
DT_V = "fp16"    # x_v, Wv
DT_PV = "bf16"   # vh_aug, expT
DT_O = "fp16"    # Wo, outT
N_WARMUP = 24    # PE p-state warmup matmuls during initial DMA wait

_cache = {}


def _np_dt(name):
    if name == "fp16":
        return np.float16
    import ml_dtypes
    return ml_dtypes.bfloat16


def _build():
    import concourse.mybir as mybir
    import concourse.tile as tile
    from concourse import bacc

    fp32 = mybir.dt.float32
    dt_qk = getattr(mybir.dt, "float16" if DT_QK == "fp16" else "bfloat16")
    dt_v = getattr(mybir.dt, "float16" if DT_V == "fp16" else "bfloat16")
    dt_pv = getattr(mybir.dt, "float16" if DT_PV == "fp16" else "bfloat16")
    dt_o = getattr(mybir.dt, "float16" if DT_O == "fp16" else "bfloat16")
    dt_out = mybir.dt.float16

    nc = bacc.Bacc("TRN2", target_bir_lowering=False, debug=False,
                   num_devices=N_CORES)

    xqT = nc.dram_tensor("xqT", [D, S], dt_qk, kind="ExternalInput").ap()
    xkT = nc.dram_tensor("xkT", [D, S], dt_qk, kind="ExternalInput").ap()
    xvT = nc.dram_tensor("xvT", [D, S], dt_v, kind="ExternalInput").ap()
    wqT = nc.dram_tensor("wqT", [D, FEAT], dt_qk, kind="ExternalInput").ap()
    wkT = nc.dram_tensor("wkT", [D, FEAT], dt_qk, kind="ExternalInput").ap()
    wvT = nc.dram_tensor("wvT", [D, FEAT], dt_v, kind="ExternalInput").ap()
    woT = nc.dram_tensor("woT", [FEAT, D], dt_o, kind="ExternalInput").ap()
    bq2 = nc.dram_tensor("bq2", [FEAT, 1], fp32, kind="ExternalInput").ap()
    bk2 = nc.dram_tensor("bk2", [FEAT, 1], fp32, kind="ExternalInput").ap()
    out_d = nc.dram_tensor("partialT", [D, S], dt_out,
                           kind="ExternalOutput").ap()

    xq_r = xqT.rearrange("(t p) s -> p t s", p=128)
    xk_r = xkT.rearrange("(t p) s -> p t s", p=128)
    xv_r = xvT.rearrange("(t p) s -> p t s", p=128)

    with tile.TileContext(nc) as tc:
        def chain(inst, key):
            # stagger DMA ring groups: rings within a group run in parallel
            # (full HBM bw); later groups start only after the prior group's
            # lane finishes, so early tensors land first.
            try:
                tc.chain_iter_dep(key, inst)
            except Exception:
                pass

        with (
            tc.tile_pool(name="win", bufs=1) as win,
            tc.tile_pool(name="big", bufs=4) as big,
            tc.tile_pool(name="proj", bufs=1) as proj,
            tc.tile_pool(name="pout", bufs=4) as pout,
            tc.tile_pool(name="pnrm", bufs=2) as pnrm,
            tc.tile_pool(name="pp", bufs=2, space="PSUM") as pp,
            tc.tile_pool(name="ps2", bufs=2, space="PSUM") as ps2,
            tc.tile_pool(name="pspv", bufs=2, space="PSUM") as pspv,
        ):
            # ---- DMA: biases unchained (tiny), then one chained stream in
            # consumption order; each ring gets full bandwidth in turn ----
            wq3 = win.tile([128, NDT, FEAT], dt_qk, tag="wq")
            wk3 = win.tile([128, NDT, FEAT], dt_qk, tag="wk")
            wv3 = win.tile([128, NDT, FEAT], dt_v, tag="wv")
            wo3 = win.tile([128, 2, D], dt_o, tag="wo")
            bq3 = win.tile([128, 2, 1], fp32, tag="bq")
            bk3 = win.tile([128, 2, 1], fp32, tag="bk")
            nc.sync.dma_start(bk3[:], bk2.rearrange("(t p) o -> p t o", p=128))
            nc.sync.dma_start(bq3[:], bq2.rearrange("(t p) o -> p t o", p=128))

            xk3 = big.tile([128, NDT, S], dt_qk, tag="big")
            xq3 = big.tile([128, NDT, S], dt_qk, tag="big")
            xv3 = big.tile([128, NDT, S], dt_v, tag="big")

            # group 1 (parallel rings): qk weights + all of xk
            nc.sync.dma_start(wk3[:], wkT.rearrange("(t p) f -> p t f", p=128))
            nc.sync.dma_start(wq3[:], wqT.rearrange("(t p) f -> p t f", p=128))
            for lane, t0 in (("a", 0), ("b", 4)):
                i = nc.sync.dma_start(xk3[:, t0:t0 + 4, :],
                                      xk_r[:, t0:t0 + 4, :])
                chain(i, lane)
            # group 2: xq, after xk
            for lane, t0 in (("a", 0), ("b", 4)):
                i = nc.sync.dma_start(xq3[:, t0:t0 + 4, :],
                                      xq_r[:, t0:t0 + 4, :])
                chain(i, lane)
            # group 3: xv + v/o weights, after xq
            for lane, t0 in (("a", 0), ("b", 4)):
                i = nc.sync.dma_start(xv3[:, t0:t0 + 4, :],
                                      xv_r[:, t0:t0 + 4, :])
                chain(i, lane)
            chain(nc.sync.dma_start(
                wv3[:], wvT.rearrange("(t p) f -> p t f", p=128)), "a")
            chain(nc.sync.dma_start(
                wo3[:], woT.rearrange("(t p) j -> p t j", p=128)), "b")

            # ---- persistent intermediates ----
            qh3 = proj.tile([128, 2, S], dt_qk, tag="qh")   # pair-packed
            kh3 = proj.tile([128, 2, S], dt_qk, tag="kh")
            vha = proj.tile([128, NKT, HL, DK + 1], dt_pv, tag="vha")
            ot3 = proj.tile([128, 2, S], dt_o, tag="outT")
            nc.gpsimd.memset(vha[:, :, :, DK], 1.0)  # ones col -> denominators

            # ---- PE p-state warmup + ACT exp-table preload while the first
            # DMAs land: matmuls on a zeroed tile into one psum scratch ----
            wdum = win.tile([128, 512], dt_qk, tag="wdum")
            junk = win.tile([128, 512], fp32, tag="junk")
            nc.gpsimd.memset(wdum[:], 0.0)
            nc.scalar.activation(junk[0:1, 0:1], wdum[0:1, 0:1],
                                 mybir.ActivationFunctionType.Exp, scale=1.0)
            wu = pp.tile([128, 512], fp32, tag="acc")
            for i in range(N_WARMUP):
                nc.tensor.matmul(wu[:], wdum[:, 0:128], wdum[:],
                                 start=(i == 0), stop=(i == N_WARMUP - 1))
            nc.vector.tensor_copy(junk[:], wu[:])

            # ---- projections: 2 psum accumulators per pass, kt-interleaved
            # so matmuls chase the chunked x DMAs ----
            def qk_pass(x3, w3, b3, dst, m, nn):
                accs = [pp.tile([128, 512], fp32, tag="acc", name=f"acc{n}")
                        for n in nn]
                for kt in range(NDT):
                    for a, n in zip(accs, nn):
                        nc.tensor.matmul(
                            a[:], w3[:, kt, m * 128:(m + 1) * 128],
                            x3[:, kt, n * 512:(n + 1) * 512],
                            start=(kt == 0), stop=(kt == NDT - 1))
                for a, n in zip(accs, nn):
                    nc.vector.tensor_scalar_add(
                        dst[:, m, n * 512:(n + 1) * 512], a[:], b3[:, m, :])

            def v_proj():
                for st in range(NKT):
                    ps = pp.tile([128, 512], fp32, tag="acc")
                    for kt in range(NDT):
                        nc.tensor.matmul(
                            ps[:, 0:256], xv3[:, kt, st * 128:(st + 1) * 128],
                            wv3[:, kt, :],
                            start=(kt == 0), stop=(kt == NDT - 1))
                    nc.vector.tensor_copy(vha[:, st, :, 0:DK], ps[:, 0:256])

            def attn_scores(qt, hp, e2u):
                for kt in range(NKT):
                    s2 = ps2.tile([128, 1024], fp32, tag="s2")
                    nc.tensor.matmul(
                        s2[:, 0:512],
                        kh3[0:64, hp, kt * 128:(kt + 1) * 128],
                        qh3[0:64, hp, qt * 512:(qt + 1) * 512],
                        start=True, stop=True)
                    nc.tensor.matmul(
                        s2[:, 512:1024],
                        kh3[64:128, hp, kt * 128:(kt + 1) * 128],
                        qh3[64:128, hp, qt * 512:(qt + 1) * 512],
                        start=True, stop=True)
                    nc.scalar.activation(
                        e2u[:, kt, :], s2[:],
                        mybir.ActivationFunctionType.Exp, scale=0.125)

            def attn_pv(qt, hp, e2u):
                pva = pspv.tile([DK + 1, 512], fp32, tag="pv")
                pvb = pspv.tile([DK + 1, 512], fp32, tag="pv")
                for kt in range(NKT):
                    nc.tensor.matmul(
                        pva[:], vha[:, kt, 2 * hp, :], e2u[:, kt, 0:512],
                        start=(kt == 0), stop=(kt == NKT - 1))
                    nc.tensor.matmul(
                        pvb[:], vha[:, kt, 2 * hp + 1, :],
                        e2u[:, kt, 512:1024],
                        start=(kt == 0), stop=(kt == NKT - 1))
                for pv, half in ((pva, 0), (pvb, 1)):
                    # copy the whole accumulator to SBUF first: frees the
                    # PSUM bank for the next unit's PV in ~0.7us instead of
                    # after the full normalize chain; DVE cost is the same
                    # (free-size bound) and custom DVE ops need SBUF anyway.
                    pvs = pnrm.tile([DK + 1, 512], fp32, tag="pvs")
                    nc.vector.tensor_copy(pvs[:], pv[:])
                    # custom-DVE recip needs a base-partition-0 input tile
                    srow = pnrm.tile([1, 512], fp32, tag="srow")
                    nc.vector.tensor_copy(srow[:], pvs[DK:DK + 1, :])
                    inv = pnrm.tile([1, 512], fp32, tag="inv")
                    nc.vector.reciprocal_approx_fast(inv[:], srow[:])
                    invb = pnrm.tile([64, 512], fp32, tag="invb")
                    nc.gpsimd.partition_broadcast(invb[:], inv[:])
                    nc.vector.tensor_tensor(
                        ot3[half * 64:(half + 1) * 64, hp,
                            qt * 512:(qt + 1) * 512],
                        pvs[0:DK, :], invb[:], mybir.AluOpType.mult)

            def oproj(qt):
                for jt in range(NDT):
                    ps = pp.tile([128, 512], fp32, tag="acc")
                    for m in range(2):
                        nc.tensor.matmul(
                            ps[:], wo3[:, m, jt * 128:(jt + 1) * 128],
                            ot3[:, m, qt * 512:(qt + 1) * 512],
                            start=(m == 0), stop=(m == 1))
                    po = pout.tile([128, 512], dt_out, tag="po")
                    nc.vector.tensor_copy(po[:], ps[:])
                    nc.sync.dma_start(
                        out_d[jt * 128:(jt + 1) * 128,
                              qt * 512:(qt + 1) * 512], po[:])

            def e2tile(name):
                return big.tile([128, NKT, 1024], dt_pv, tag="big", name=name)

            # ---- emission order == per-engine execution order ----
            for m in range(2):                       # k-proj (all pairs)
                for nn in ((0, 1), (2, 3)):
                    qk_pass(xk3, wk3, bk3, kh3, m, nn)
            qk_pass(xq3, wq3, bq3, qh3, 0, (0,))     # q-proj heads only
            qk_pass(xq3, wq3, bq3, qh3, 1, (0,))
            e00 = e2tile("e00")
            attn_scores(0, 0, e00)                   # ACT starts here
            e01 = e2tile("e01")
            attn_scores(0, 1, e01)
            for m in range(2):                       # q-proj remainder
                qk_pass(xq3, wq3, bq3, qh3, m, (1, 2))
                qk_pass(xq3, wq3, bq3, qh3, m, (3,))
            v_proj()
            attn_pv(0, 0, e00)
            e10 = e2tile("e10")
            attn_scores(1, 0, e10)
            attn_pv(0, 1, e01)
            oproj(0)
            prev = {(1, 0): e10}
            for qt in range(1, NQT):
                e_b = e2tile(f"e{qt}1")
                attn_scores(qt, 1, e_b)
                attn_pv(qt, 0, prev[(qt, 0)])
                if qt < NQT - 1:
                    e_a = e2tile(f"e{qt + 1}0")
                    attn_scores(qt + 1, 0, e_a)
                    prev[(qt + 1, 0)] = e_a
                attn_pv(qt, 1, e_b)
                oproj(qt)

    nc.compile()
    return nc


def kernel(q, k, v, Wq, bq, Wk, bk, Wv, bv, Wo, bo, _trace=False):
    from concourse import bass_utils

    if "nc" not in _cache:
        _cache["nc"] = _build()
    nc = _cache["nc"]

    q = np.asarray(q, np.float32)
    k = np.asarray(k, np.float32)
    v = np.asarray(v, np.float32)
    Wq = np.asarray(Wq, np.float32)
    Wk = np.asarray(Wk, np.float32)
    Wv = np.asarray(Wv, np.float32)
    Wo = np.asarray(Wo, np.float32)
    bq = np.asarray(bq, np.float32)
    bk = np.asarray(bk, np.float32)
    bv = np.asarray(bv, np.float32)
    bo = np.asarray(bo, np.float32)

    d_qk, d_v, d_o = _np_dt(DT_QK), _np_dt(DT_V), _np_dt(DT_O)
    xT = {}
    for b in range(B):
        xT[("q", b)] = np.ascontiguousarray(q[b].T).astype(d_qk)
        xT[("k", b)] = np.ascontiguousarray(k[b].T).astype(d_qk)
        xT[("v", b)] = np.ascontiguousarray(v[b].T).astype(d_v)
    wT = {}
    for g in range(HG):
        sl = slice(g * FEAT, (g + 1) * FEAT)
        wT[("q", g)] = np.ascontiguousarray(Wq[sl, :].T).astype(d_qk)
        wT[("k", g)] = np.ascontiguousarray(Wk[sl, :].T).astype(d_qk)
        wT[("v", g)] = np.ascontiguousarray(Wv[sl, :].T).astype(d_v)
        wT[("o", g)] = np.ascontiguousarray(Wo[:, sl].T).astype(d_o)

    in_maps = []
    for c in range(N_CORES):
        b, g = divmod(c, HG)
        sl = slice(g * FEAT, (g + 1) * FEAT)
        in_maps.append({
            "xqT": xT[("q", b)], "xkT": xT[("k", b)], "xvT": xT[("v", b)],
            "wqT": wT[("q", g)], "wkT": wT[("k", g)], "wvT": wT[("v", g)],
            "woT": wT[("o", g)],
            "bq2": np.ascontiguousarray(bq[sl]).reshape(FEAT, 1),
            "bk2": np.ascontiguousarray(bk[sl]).reshape(FEAT, 1),
        })

    kwargs = {}
    if _trace:
        _install_profile_shim()
        kwargs = dict(trace=True, trace_cores=list(range(N_CORES)))
    res = bass_utils.run_bass_kernel_spmd(
        nc, in_maps, core_ids=list(range(N_CORES)), **kwargs)
    _cache["last_results"] = res

    final_bias = (Wo @ bv + bo).astype(np.float32)  # attn rows sum to 1
    out = np.empty((B, S, D), np.float32)
    for b in range(B):
        acc = res.results[b * HG]["partialT"].astype(np.float32)
        for g in range(1, HG):
            acc += res.results[b * HG + g]["partialT"].astype(np.float32)
        out[b] = acc.T + final_bias
    return out


def _install_profile_shim():
    """Provide antenv.axon_hooks so trace=True works under axon."""
    import sys
    import types

    import antenv

    if "antenv.axon_hooks" in sys.modules:
        return
    mod = types.ModuleType("antenv.axon_hooks")
    mod._hook = None
    mod.set_axon_ntff_profile_hook = lambda h: setattr(mod, "_hook", h)
    mod.get_axon_ntff_profile_hook = lambda: mod._hook
    sys.modules["antenv.axon_hooks"] = mod
    antenv.axon_hooks = mod
    try:
        from trn_agent_boot.trn_boot import _ntff_profile_via_ctypes
        mod.set_axon_ntff_profile_hook(
            _ntff_profile_via_ctypes("/opt/axon/libaxon_pjrt.so"))
    except Exception:
        pass
